# revision 20
# baseline (speedup 1.0000x reference)
"""Gabor layer Trainium2 kernel (v3: packed planes).

Per gabor g and pixel (x,y): amp[g,c] * exp(E) * cos(S + phase[g,c]).
cos(S+p) = cos(p)cos(S) - sin(p)sin(S) turns the channel sum over g into
matmuls over gauss*cos(S) / gauss*sin(S) planes (contraction = gabors).

All elementwise engine costs scale with the free (pixel) axis only, so the
partition axis is free parallelism. Each 64-row strip is culled per column
half; with kL,kR <= 64 two tiles (one left-half, one right-half) pack into
one 128-partition plane, halving every exp/sin/wrap/multiply:
  plane pl = (row_block, col_block<8): partitions 0:64 carry the left
  tile's gabors, 64:128 the right tile's (tile tR = tL + 8).

Per plane:  E = [WEh;WEl]^T @ feat12     (1 f32r matmul; integer tile-local
            features [dj,di,1,dj2,di2,dj*di] are exact in f32r, weights
            hi/lo split; f32r runs 1 cycle/row vs 4 for fp32 and the PE
            product is exact for pre-rounded inputs)
            S = [Ah;Bh;Al;Bl]^T @ onehot96   (1 bf16 matmul, K=96)
            gauss = Exp(E) fp16; w1 = wrap(S); w2 = wrap(S+pi/2) (DVE);
            ss,cs = Sin(w1),Sin(w2) fp16; p1 = cs*gauss, p2 = ss*gauss
            (fp16 DVE 2x mode)
Output: 4 logical tiles accumulate into ONE PSUM bank at partition offsets
0/32/64/96 (tile_position column tiling, one accumulation group per bank),
then one 512-cycle DVE copy + 2 DMAs per quad.

Two global phases (all Exps, then all Sins) keep the Exp/Sin activation
tables from thrashing: they live in different hardware table sets and each
swap costs 1.3us (the v2 interleaved phasing measured 36 loads = 46us).

Sharding: 8 cores x 64-row strips, no collectives; clamp + reassembly on
host. Falls back to the v2 per-tile program if a column half keeps > 64
gabors (not the case for the reference inputs: kL<=55, kR<=64).
"""

import os
import sys

import numpy as np

for _p in ("/opt/trn_rl_repo",):
    if os.path.isdir(_p) and _p not in sys.path:
        sys.path.append(_p)

H = W = 512
G = 256
NCORES = 8
SH = H // NCORES      # strip rows per core
TR, TC = 16, 32       # tile rows x cols
N = TR * TC           # 512 pixels per tile
TPR = W // TC         # tiles per strip row = 16
NT = (SH // TR) * TPR # tiles per core = 64
NPL = NT // 2         # packed planes per core = 32
KS = 2 * (TR + TC)    # one-hot rows: [rowhi, colhi, rowlo, collo] = 96
PI = float(np.pi)
CULL_THR = 1e-7
SIN_FLOOR_MS = 0.027   # ACT floor for the Sin phase (past the last Exp)

_PROGRAMS = {}


def _build_program_packed():
    from concourse import bacc, mybir, tile

    f32 = mybir.dt.float32
    f32r = mybir.dt.float32r
    bf16 = mybir.dt.bfloat16
    f16 = mybir.dt.float16
    Act = mybir.ActivationFunctionType

    nc = bacc.Bacc("TRN2", target_bir_lowering=False, debug=False,
                   num_devices=NCORES)

    featd = nc.dram_tensor("feat", [12, N], f32r, kind="ExternalInput")
    ohd = nc.dram_tensor("onehot", [KS, N], bf16, kind="ExternalInput")
    wed = nc.dram_tensor("we", [12, NPL, 128], f32r, kind="ExternalInput")
    wsd = nc.dram_tensor("ws", [NPL // 8, KS, 8 * 128], bf16,
                         kind="ExternalInput")
    abd = nc.dram_tensor("ab", [128, 6], f16, kind="ExternalInput")
    outd = nc.dram_tensor("out", [2, 3, NPL, N], f32, kind="ExternalOutput")

    with tile.TileContext(nc) as tc:
        with (
            tc.tile_pool(name="io", bufs=1) as iop,
            tc.tile_pool(name="gauss", bufs=3) as gp,
            tc.tile_pool(name="trig", bufs=3) as trigp,
            tc.tile_pool(name="prod", bufs=3) as pp,
            tc.tile_pool(name="mms", bufs=4, space="PSUM") as mmsp,
            tc.tile_pool(name="mme", bufs=2, space="PSUM") as mmep,
        ):
            ab_sb = iop.tile([128, 6], f16, tag="ab")
            nc.scalar.dma_start(out=ab_sb[:], in_=abd[:])
            oh_sb = iop.tile([KS, N], bf16, tag="oh")
            nc.scalar.dma_start(out=oh_sb[:], in_=ohd[:])
            ft_sb = iop.tile([12, N], f32r, tag="ft")
            nc.scalar.dma_start(out=ft_sb[:], in_=featd[:])
            we = iop.tile([12, NPL, 128], f32r, tag="we")
            nc.scalar.dma_start(out=we[:], in_=wed[:])
            # PE interleaves wrap-gated mS singles with free-running mE
            # pairs; DVE wraps stream from ~4us (the critical chain); ACT
            # runs [all Exp-pairs][all Sins] - Sins floored past the last
            # Exp (Copy needs no table load). po tiles share the mS PSUM
            # ring (mS banks are dead once wrapped).
            wqs = []
            gts = []
            w1q = gq = mEp = None
            for pl in range(NPL):
                if pl == 0:
                    wsc = iop.tile([KS, 8, 128], bf16, tag="ws", bufs=2)
                    nc.sync.dma_start(out=wsc[:, 0:2, :],
                                      in_=wsd[0, :, 0:2 * 128])
                    nc.sync.dma_start(out=wsc[:, 2:8, :],
                                      in_=wsd[0, :, 2 * 128:])
                elif pl % 8 == 0:
                    wsc = iop.tile([KS, 8, 128], bf16, tag="ws", bufs=2)
                    nc.sync.dma_start(out=wsc[:], in_=wsd[pl // 8])
                mS = mmsp.tile([128, N], f32, tag="mS", name="mS")
                nc.tensor.matmul(mS[:], wsc[:, pl % 8, :], oh_sb[:],
                                 start=True, stop=True)
                # two mE matmuls per mS: all Exps are done by mid-kernel
                for e in (2 * pl, 2 * pl + 1):
                    if e >= NPL:
                        continue
                    if e % 2 == 0:
                        mEp = mmep.tile([128, 2, N], f32, tag="mE",
                                        name="mE")
                    nc.tensor.matmul(mEp[:, e % 2, :], we[:, e, :],
                                     ft_sb[:], start=True, stop=True)
                    if e % 4 == 0:
                        gq = gp.tile([128, 4, N], f16, tag="g",
                                     name="gauss", bufs=NPL // 4 + 1)
                        gts.append(gq)
                    if e % 2 == 1:
                        nc.scalar.activation(gq[:, (e % 4) - 1:(e % 4) + 1],
                                             mEp[:], Act.Exp)
                if pl % 4 == 0:
                    w1q = trigp.tile([128, 4, N], f16, tag="w1", name="w1",
                                     bufs=NPL // 4 + 1)
                nc.vector.add_range_wrap(w1q[:, pl % 4], mS[:],
                                         0.0, PI, 2.0 * PI)
                if pl % 4 == 3:
                    w2q = trigp.tile([128, 4, N], f16, tag="w2", name="w2",
                                     bufs=NPL // 4 + 1)
                    nc.vector.add_range_wrap(w2q[:], w1q[:],
                                             PI / 2, PI, 2.0 * PI)
                    wqs.append((w1q, w2q))

            for i in range(NPL // 4):
                w1q, w2q = wqs[i]
                gq = gts[i]
                with tc.tile_wait_until(SIN_FLOOR_MS):
                    ssq = trigp.tile([128, 4, N], f16, tag="ss", name="ss")
                    nc.scalar.activation(ssq[:], w1q[:], Act.Sin)
                    csq = trigp.tile([128, 4, N], f16, tag="cs", name="cs")
                    nc.scalar.activation(csq[:], w2q[:], Act.Sin)
                for hh in range(2):       # pair within quad
                    p1p = pp.tile([128, 2, N], f16, tag="p1", name="p1")
                    nc.vector.tensor_mul(p1p[:], gq[:, 2 * hh:2 * hh + 2],
                                         csq[:, 2 * hh:2 * hh + 2])
                    p2p = pp.tile([128, 2, N], f16, tag="p2", name="p2")
                    nc.vector.tensor_mul(p2p[:], gq[:, 2 * hh:2 * hh + 2],
                                         ssq[:, 2 * hh:2 * hh + 2])
                    _emit_reduce(nc, mmsp, pp, ab_sb, outd, p1p, p2p,
                                 0, 2 * i + hh)

    nc.compile()
    return nc


def _emit_reduce(nc, accp, pp, ab_sb, outd, p1q, p2q, hh, q):
    """2 logical tiles per PSUM bank at partition offsets 0/32 (base 96
    is illegal - PE quadrant 3), bank h = plane h of the pair; one
    accumulation group per (bank, region). PSUM->SBUF copies alternate
    between the Scalar and Vector engines to balance load."""
    from concourse import mybir
    f32 = mybir.dt.float32
    N_ = p1q.shape[-1]
    pos = [accp.tile([128, N_], f32, tag="mS", name="po") for _ in range(2)]
    for h in range(2):        # plane within pair = bank
        for s in range(2):    # side: 0=left(K 0:64) 1=right
            ks, co = s * 64, s * 32
            for pi_, (src, acol) in enumerate(((p1q, 0), (p2q, 3))):
                nc.tensor.matmul(
                    pos[h][co:co + 3, :],
                    ab_sb[ks:ks + 64, acol:acol + 3],
                    src[ks:ks + 64, h, :],
                    start=(pi_ == 0), stop=(pi_ == 1),
                    skip_group_check=True,
                )
    ob = pp.tile([128, 2, N_], f32, tag="ob", name="ob")
    for h in range(2):
        if (q + h) % 2 == 0:
            nc.scalar.copy(ob[:, h, :], pos[h][:])
        else:
            nc.vector.tensor_copy(ob[:, h, :], pos[h][:])
    nc.sync.dma_start(out=outd[0, :, 2 * q:2 * q + 2, :], in_=ob[0:3, :, :])
    nc.sync.dma_start(out=outd[1, :, 2 * q:2 * q + 2, :], in_=ob[32:35, :, :])


def _wrap(x):
    return np.mod(x + np.pi, 2.0 * np.pi) - np.pi


def _to_f32r(a):
    b = np.ascontiguousarray(a, np.float32).view(np.uint32)
    r = (b + np.uint32(0x7FF) + ((b >> np.uint32(12)) & np.uint32(1))) \
        & np.uint32(0xFFFFF000)
    return r.view(np.float32)


def _to_bf16(a):
    import ml_dtypes
    return np.ascontiguousarray(a.astype(ml_dtypes.bfloat16))


def _fold_params(inputs):
    u = np.clip(np.asarray(inputs["u"], np.float64), -1, 1)
    v = np.clip(np.asarray(inputs["v"], np.float64), -1, 1)
    th = np.clip(np.asarray(inputs["theta"], np.float64), -2, 2) * (2 * np.pi)
    sig = np.clip(np.asarray(inputs["rel_sigma"], np.float64), 0.001, 1.0)
    rf = np.clip(np.asarray(inputs["rel_freq"], np.float64), -5, 5)
    gam = np.clip(np.asarray(inputs["gamma"], np.float64), 0.0001, 1.0)
    psi = np.clip(np.asarray(inputs["psi"], np.float64), -1, 1)
    amp = np.clip(np.asarray(inputs["amplitude"], np.float64), 0, 1)
    cr, sr = np.cos(th), np.sin(th)
    return dict(
        u=u, v=v, cr=cr, sr=sr,
        cx=-(cr * u + sr * v), cy=sr * u - cr * v,
        p=1.0 / (2.0 * sig * sig), q=1.0 / (2.0 * gam * gam),
        freq=2 * np.pi / np.exp(rf),
        alpha=amp * np.cos(psi * 2 * np.pi),
        beta=-amp * np.sin(psi * 2 * np.pi),
        amp=amp,
    )


def _keeps(P, gx, gy, rows, cols):
    """Exact per-pixel cull: keep gabors whose max E over the region
    clears the contribution threshold."""
    ampmax = P["amp"].max(1)
    elim = np.log(np.maximum(CULL_THR / np.maximum(ampmax, 1e-30),
                             1e-300)) - 1.0
    crf = P["cr"].astype(np.float32)[:, None]
    srf = P["sr"].astype(np.float32)[:, None]
    pf = P["p"].astype(np.float32)[:, None]
    qf = P["q"].astype(np.float32)[:, None]
    Xs = np.asarray(gx[rows][:, cols], np.float32).ravel()[None, :]
    Ys = np.asarray(gy[rows][:, cols], np.float32).ravel()[None, :]
    dx = Xs - P["u"].astype(np.float32)[:, None]
    dy = Ys - P["v"].astype(np.float32)[:, None]
    xr = dx * crf + dy * srf
    yr = dy * crf - dx * srf
    quad = xr * xr * pf
    quad += yr * yr * qf
    Em = -quad.min(1)
    return np.flatnonzero(Em >= elim)


def _tile_geometry(gx, gy):
    """Tile-major grids and per-tile affine centers/steps."""
    Xt = gx.reshape(H // TR, TR, W // TC, TC).transpose(0, 2, 1, 3).reshape(-1, N)
    Yt = gy.reshape(H // TR, TR, W // TC, TC).transpose(0, 2, 1, 3).reshape(-1, N)
    hx = Xt[:, 1] - Xt[:, 0]
    hy = Yt[:, TC] - Yt[:, 0]
    Xc = Xt[:, TR // 2 * TC + TC // 2]
    Yc = Yt[:, TR // 2 * TC + TC // 2]
    yrow = Yt.reshape(-1, TR, TC)[:, :, 0]
    xcol = Xt.reshape(-1, TR, TC)[:, 0, :]
    return Xc, Yc, hx, hy, yrow, xcol


def _tile_tables(P, keep, tiles, Xc, Yc, hx, hy, yrow, xcol):
    """WE [6, n, k], A [n, k, TR], B [n, k, TC] for the given gabor subset
    over the given tile indices (float64)."""
    crk, srk = P["cr"][keep], P["sr"][keep]
    cxk, cyk = P["cx"][keep], P["cy"][keep]
    pk, qk = P["p"][keep], P["q"][keep]
    fk = P["freq"][keep]
    XcT = Xc[tiles][:, None]
    YcT = Yc[tiles][:, None]
    hxT = hx[tiles][:, None]
    hyT = hy[tiles][:, None]
    cxt = XcT * crk[None, :] + YcT * srk[None, :] + cxk[None, :]
    cyt = -XcT * srk[None, :] + YcT * crk[None, :] + cyk[None, :]
    a1 = hxT * crk[None, :]
    a2 = hyT * srk[None, :]
    b1 = -hxT * srk[None, :]
    b2 = hyT * crk[None, :]
    n, k = cxt.shape
    WE = np.empty((6, n, k))
    WE[0] = -2.0 * (pk * cxt * a1 + qk * cyt * b1)
    WE[1] = -2.0 * (pk * cxt * a2 + qk * cyt * b2)
    WE[2] = -(pk * cxt * cxt + qk * cyt * cyt)
    WE[3] = -(pk * a1 * a1 + qk * b1 * b1)
    WE[4] = -(pk * a2 * a2 + qk * b2 * b2)
    WE[5] = -2.0 * (pk * a1 * a2 + qk * b1 * b2)
    A = _wrap(fk[None, :, None] * srk[None, :, None]
              * (yrow[tiles][:, None, :] - YcT[:, :, None]))
    Bt = _wrap(fk[None, :, None] * crk[None, :, None]
               * (xcol[tiles][:, None, :] - XcT[:, :, None])
               + (fk[None, :] * cxt)[:, :, None])
    return WE, A, Bt


def _host_arrays_packed(inputs, P, gx, gy, keepLR):
    ii, jj = np.divmod(np.arange(N), TC)
    di = (ii - TR // 2).astype(np.float64)
    dj = (jj - TC // 2).astype(np.float64)
    feat6 = np.stack([dj, di, np.ones_like(dj), dj * dj, di * di, dj * di], 0)
    feat12 = np.concatenate([feat6, feat6], 0).astype(np.float32)

    onehot = np.zeros((KS, N), np.float32)
    onehot[ii, np.arange(N)] = 1.0
    onehot[TR + jj, np.arange(N)] = 1.0
    onehot[TR + TC:] = onehot[:TR + TC]

    Xc, Yc, hx, hy, yrow, xcol = _tile_geometry(gx, gy)

    # plane pl = r*8+cb -> left tile r*16+cb, right tile r*16+cb+8
    rr = np.arange(NPL) // 8
    cc = np.arange(NPL) % 8

    in_maps = []
    for core in range(NCORES):
        keepL, keepR = keepLR[core]
        base = core * NT
        tilesL = base + rr * 16 + cc
        tilesR = tilesL + 8

        we12 = np.zeros((12, NPL, 128), np.float32)
        ws = np.zeros((KS, NPL, 128))
        AB = np.zeros((128, 6))
        for side, keep, tiles in (
            (0, keepL, tilesL), (1, keepR, tilesR)
        ):
            k = len(keep)
            o = side * 64
            WE, A, Bt = _tile_tables(P, keep, tiles, Xc, Yc, hx, hy,
                                     yrow, xcol)
            WEh = _to_f32r(WE)
            WEl = _to_f32r(WE - WEh)
            we12[0:6, :, o:o + k] = WEh
            we12[6:12, :, o:o + k] = WEl
            WS = np.concatenate([A.transpose(2, 0, 1),
                                 Bt.transpose(2, 0, 1)], 0)  # [48, NPL, k]
            WSh = _to_bf16(WS).astype(np.float64)
            ws[0:48, :, o:o + k] = WSh
            ws[48:96, :, o:o + k] = WS - WSh
            AB[o:o + k, 0:3] = P["alpha"][keep]
            AB[o:o + k, 3:6] = P["beta"][keep]

        ws_cm = ws.reshape(KS, NPL // 8, 8 * 128).transpose(1, 0, 2)
        in_maps.append({
            "feat": feat12,
            "onehot": _to_bf16(onehot),
            "we": np.ascontiguousarray(we12),
            "ws": _to_bf16(ws_cm),
            "ab": AB.astype(np.float16),
        })
    return in_maps


def kernel(**inputs):
    from concourse.bass_utils import run_bass_kernel_spmd

    gx = np.asarray(inputs["grid_x"], np.float64)
    gy = np.asarray(inputs["grid_y"], np.float64)
    P = _fold_params(inputs)

    keepLR = []
    packed = True
    for core in range(NCORES):
        rows = slice(core * SH, (core + 1) * SH)
        kL = _keeps(P, gx, gy, rows, slice(0, W // 2))
        kR = _keeps(P, gx, gy, rows, slice(W // 2, W))
        if len(kL) > 64 or len(kR) > 64:
            packed = False
        keepLR.append((kL, kR))

    if not packed:
        return _kernel_unpacked(inputs)

    in_maps = _host_arrays_packed(inputs, P, gx, gy, keepLR)
    if "packed" not in _PROGRAMS:
        _PROGRAMS["packed"] = _build_program_packed()
    nc = _PROGRAMS["packed"]
    res = run_bass_kernel_spmd(nc, in_maps, list(range(NCORES)))
    out = np.empty((3, H, W), np.float32)
    for core in range(NCORES):
        r = res.results[core]["out"]              # [2, 3, NPL, N]
        # plane pl = rowblk*8+cb; side 0 -> tile col cb, side 1 -> cb+8
        arr = r.reshape(2, 3, SH // TR, 8, TR, TC)
        out[:, core * SH:(core + 1) * SH, :] = (
            arr.transpose(1, 2, 4, 0, 3, 5).reshape(3, SH, W)
        )
    np.clip(out, -1.0, 1.0, out=out)
    return out


# ---------------------------------------------------------------------------
# Fallback: v2 per-tile program (used only if a column half keeps > 64
# gabors; correct for any input).
# ---------------------------------------------------------------------------

B_FB = 8


def _build_program_unpacked(nchunk):
    from concourse import bacc, mybir, tile

    f32 = mybir.dt.float32
    f32r = mybir.dt.float32r
    bf16 = mybir.dt.bfloat16
    f16 = mybir.dt.float16
    Act = mybir.ActivationFunctionType
    Gc = 128 * nchunk
    mmbufs = 2 if nchunk == 1 else 1
    NBLK = NT // B_FB

    nc = bacc.Bacc("TRN2", target_bir_lowering=False, debug=False,
                   num_devices=NCORES)

    featd = nc.dram_tensor("feat", [12, N], f32r, kind="ExternalInput")
    ohd = nc.dram_tensor("onehot", [KS, N], bf16, kind="ExternalInput")
    wed = nc.dram_tensor("we", [12, NT, Gc], f32r, kind="ExternalInput")
    wsd = nc.dram_tensor("ws", [KS, NT, Gc], bf16, kind="ExternalInput")
    abd = nc.dram_tensor("ab", [128, nchunk * 2 * 3], f16,
                         kind="ExternalInput")
    outd = nc.dram_tensor("out", [3, NT, N], f32, kind="ExternalOutput")

    with tile.TileContext(nc) as tc:
        with (
            tc.tile_pool(name="io", bufs=1) as iop,
            tc.tile_pool(name="gauss", bufs=B_FB // 2 + 2) as gp,
            tc.tile_pool(name="trig", bufs=3) as trigp,
            tc.tile_pool(name="prod", bufs=3) as pp,
            tc.tile_pool(name="mme", bufs=mmbufs, space="PSUM") as mmep,
            tc.tile_pool(name="mms", bufs=mmbufs, space="PSUM") as mmsp,
            tc.tile_pool(name="acc", bufs=2, space="PSUM") as accp,
        ):
            ab_sb = iop.tile([128, nchunk * 2 * 3], f16, tag="ab")
            nc.sync.dma_start(out=ab_sb[:], in_=abd[:])
            oh_sb = iop.tile([KS, N], bf16, tag="oh")
            nc.sync.dma_start(out=oh_sb[:], in_=ohd[:])
            ft_sb = iop.tile([12, N], f32r, tag="ft")
            nc.sync.dma_start(out=ft_sb[:], in_=featd[:])

            for blk in range(NBLK):
                t0 = blk * B_FB
                we = iop.tile([12, B_FB, Gc], f32r, tag="we", bufs=2)
                nc.sync.dma_start(out=we[:], in_=wed[:, t0:t0 + B_FB, :])
                ws = iop.tile([KS, B_FB, Gc], bf16, tag="ws", bufs=2)
                nc.sync.dma_start(out=ws[:], in_=wsd[:, t0:t0 + B_FB, :])

                gts = []
                for t in range(B_FB):
                    mE = mmep.tile([128, nchunk, N], f32, tag="mE", name="mE")
                    for c in range(nchunk):
                        nc.tensor.matmul(
                            mE[:, c, :],
                            we[:, t, c * 128:(c + 1) * 128],
                            ft_sb[:],
                            start=True, stop=True,
                        )
                    if t % 2 == 0:
                        gpair = gp.tile([128, 2 * nchunk, N], f16, tag="g",
                                        name="gauss")
                        gts.append(gpair)
                    nc.scalar.activation(
                        gpair[:, (t % 2) * nchunk:(t % 2 + 1) * nchunk],
                        mE[:], Act.Exp)

                for t in range(B_FB):
                    mS = mmsp.tile([128, nchunk, N], f32, tag="mS", name="mS")
                    for c in range(nchunk):
                        nc.tensor.matmul(
                            mS[:, c, :],
                            ws[:, t, c * 128:(c + 1) * 128],
                            oh_sb[:],
                            start=True, stop=True,
                        )
                    if t % 2 == 0:
                        w1p = trigp.tile([128, 2 * nchunk, N], f16, tag="w1",
                                         name="w1")
                    nc.vector.add_range_wrap(
                        w1p[:, (t % 2) * nchunk:(t % 2 + 1) * nchunk],
                        mS[:], 0.0, PI, 2.0 * PI)
                    if t % 2 == 1:
                        w2p = trigp.tile([128, 2 * nchunk, N], f16, tag="w2",
                                         name="w2")
                        nc.vector.add_range_wrap(w2p[:], w1p[:],
                                                 PI / 2, PI, 2.0 * PI)
                        ssp = trigp.tile([128, 2 * nchunk, N], f16, tag="ss",
                                         name="ss")
                        nc.scalar.activation(ssp[:], w1p[:], Act.Sin)
                        csp = trigp.tile([128, 2 * nchunk, N], f16, tag="cs",
                                         name="cs")
                        nc.scalar.activation(csp[:], w2p[:], Act.Sin)

                        gpair = gts[t // 2]
                        p1p = pp.tile([128, 2 * nchunk, N], f16, tag="p1",
                                      name="p1")
                        nc.vector.tensor_mul(p1p[:], gpair[:], csp[:])
                        p2p = pp.tile([128, 2 * nchunk, N], f16, tag="p2",
                                      name="p2")
                        nc.vector.tensor_mul(p2p[:], gpair[:], ssp[:])

                        po = accp.tile([3, 2, N], f32, tag="po", name="po")
                        for hh in range(2):
                            ops = [(p1p, c) for c in range(nchunk)] + \
                                  [(p2p, c) for c in range(nchunk)]
                            for ci, (src, c) in enumerate(ops):
                                ab_col = (0 if src is p1p
                                          else 3 * nchunk) + 3 * c
                                nc.tensor.matmul(
                                    po[:, hh],
                                    ab_sb[:, ab_col:ab_col + 3],
                                    src[:, hh * nchunk + c, :],
                                    start=(ci == 0),
                                    stop=(ci == len(ops) - 1),
                                )
                        ob = pp.tile([3, 2, N], f32, tag="ob", name="ob")
                        nc.vector.tensor_copy(ob[:], po[:])
                        nc.sync.dma_start(
                            out=outd[:, t0 + t - 1:t0 + t + 1, :],
                            in_=ob[:],
                        )

    nc.compile()
    return nc


def _kernel_unpacked(inputs):
    from concourse.bass_utils import run_bass_kernel_spmd

    gx = np.asarray(inputs["grid_x"], np.float64)
    gy = np.asarray(inputs["grid_y"], np.float64)
    P = _fold_params(inputs)

    keep_lists = []
    for core in range(NCORES):
        rows = slice(core * SH, (core + 1) * SH)
        keep_lists.append(_keeps(P, gx, gy, rows, slice(0, W)))
    gmax = max(len(k) for k in keep_lists)
    nchunk = max(1, -(-gmax // 128))
    Gc = 128 * nchunk

    ii, jj = np.divmod(np.arange(N), TC)
    di = (ii - TR // 2).astype(np.float64)
    dj = (jj - TC // 2).astype(np.float64)
    feat6 = np.stack([dj, di, np.ones_like(dj), dj * dj, di * di, dj * di], 0)
    feat12 = np.concatenate([feat6, feat6], 0).astype(np.float32)

    onehot = np.zeros((KS, N), np.float32)
    onehot[ii, np.arange(N)] = 1.0
    onehot[TR + jj, np.arange(N)] = 1.0
    onehot[TR + TC:] = onehot[:TR + TC]

    Xc, Yc, hx, hy, yrow, xcol = _tile_geometry(gx, gy)

    in_maps = []
    for core in range(NCORES):
        keep = keep_lists[core]
        k = len(keep)
        tiles = np.arange(core * NT, (core + 1) * NT)
        WE, A, Bt = _tile_tables(P, keep, tiles, Xc, Yc, hx, hy, yrow, xcol)

        we12 = np.zeros((12, NT, Gc), np.float32)
        WEh = _to_f32r(WE)
        we12[0:6, :, :k] = WEh
        we12[6:12, :, :k] = _to_f32r(WE - WEh)

        ws = np.zeros((KS, NT, Gc))
        WS = np.concatenate([A.transpose(2, 0, 1), Bt.transpose(2, 0, 1)], 0)
        WSh = _to_bf16(WS).astype(np.float64)
        ws[0:48, :, :k] = WSh
        ws[48:96, :, :k] = WS - WSh

        AB = np.zeros((128, nchunk * 2 * 3))
        al = np.zeros((Gc, 3)); bt = np.zeros((Gc, 3))
        al[:k] = P["alpha"][keep]
        bt[:k] = P["beta"][keep]
        for c in range(nchunk):
            AB[:, 3 * c:3 * c + 3] = al[c * 128:(c + 1) * 128]
            off = 3 * (nchunk + c)
            AB[:, off:off + 3] = bt[c * 128:(c + 1) * 128]

        in_maps.append({
            "feat": feat12,
            "onehot": _to_bf16(onehot),
            "we": np.ascontiguousarray(we12),
            "ws": _to_bf16(ws),
            "ab": AB.astype(np.float16),
        })

    key = ("unpacked", nchunk)
    if key not in _PROGRAMS:
        _PROGRAMS[key] = _build_program_unpacked(nchunk)
    nc = _PROGRAMS[key]
    res = run_bass_kernel_spmd(nc, in_maps, list(range(NCORES)))
    out = np.empty((3, H, W), np.float32)
    for core in range(NCORES):
        r = res.results[core]["out"]              # [3, NT, N]
        out[:, core * SH:(core + 1) * SH, :] = (
            r.reshape(3, SH // TR, TPR, TR, TC)
             .transpose(0, 1, 3, 2, 4)
             .reshape(3, SH, W)
        )
    np.clip(out, -1.0, 1.0, out=out)
    return out


# revision 21
# speedup vs baseline: 1.0266x; 1.0266x over previous
"""Gabor layer Trainium2 kernel (v3: packed planes).

Per gabor g and pixel (x,y): amp[g,c] * exp(E) * cos(S + phase[g,c]).
cos(S+p) = cos(p)cos(S) - sin(p)sin(S) turns the channel sum over g into
matmuls over gauss*cos(S) / gauss*sin(S) planes (contraction = gabors).

All elementwise engine costs scale with the free (pixel) axis only, so the
partition axis is free parallelism. Each 64-row strip is culled per column
half; with kL,kR <= 64 two tiles (one left-half, one right-half) pack into
one 128-partition plane, halving every exp/sin/wrap/multiply:
  plane pl = (row_block, col_block<8): partitions 0:64 carry the left
  tile's gabors, 64:128 the right tile's (tile tR = tL + 8).

Per plane:  E = [WEh;WEl]^T @ feat12     (1 f32r matmul; integer tile-local
            features [dj,di,1,dj2,di2,dj*di] are exact in f32r, weights
            hi/lo split; f32r runs 1 cycle/row vs 4 for fp32 and the PE
            product is exact for pre-rounded inputs)
            S = [Ah;Bh;Al;Bl]^T @ onehot96   (1 bf16 matmul, K=96)
            gauss = Exp(E) fp16; w1 = wrap(S); w2 = wrap(S+pi/2) (DVE);
            ss,cs = Sin(w1),Sin(w2) fp16; p1 = cs*gauss, p2 = ss*gauss
            (fp16 DVE 2x mode)
Output: 4 logical tiles accumulate into ONE PSUM bank at partition offsets
0/32/64/96 (tile_position column tiling, one accumulation group per bank),
then one 512-cycle DVE copy + 2 DMAs per quad.

Two global phases (all Exps, then all Sins) keep the Exp/Sin activation
tables from thrashing: they live in different hardware table sets and each
swap costs 1.3us (the v2 interleaved phasing measured 36 loads = 46us).

Sharding: 8 cores x 64-row strips, no collectives; clamp + reassembly on
host. Falls back to the v2 per-tile program if a column half keeps > 64
gabors (not the case for the reference inputs: kL<=55, kR<=64).
"""

import os
import sys

import numpy as np

for _p in ("/opt/trn_rl_repo",):
    if os.path.isdir(_p) and _p not in sys.path:
        sys.path.append(_p)

H = W = 512
G = 256
NCORES = 8
SH = H // NCORES      # strip rows per core
TR, TC = 16, 32       # tile rows x cols
N = TR * TC           # 512 pixels per tile
TPR = W // TC         # tiles per strip row = 16
NT = (SH // TR) * TPR # tiles per core = 64
NPL = NT // 2         # packed planes per core = 32
KS = 2 * (TR + TC)    # one-hot rows: [rowhi, colhi, rowlo, collo] = 96
PI = float(np.pi)
CULL_THR = 1e-7
SIN_FLOOR_MS = 0.027   # ACT floor for the Sin phase (past the last Exp)

_PROGRAMS = {}


def _build_program_packed():
    from concourse import bacc, mybir, tile

    f32 = mybir.dt.float32
    f32r = mybir.dt.float32r
    bf16 = mybir.dt.bfloat16
    f16 = mybir.dt.float16
    Act = mybir.ActivationFunctionType

    nc = bacc.Bacc("TRN2", target_bir_lowering=False, debug=False,
                   num_devices=NCORES)

    featd = nc.dram_tensor("feat", [12, N], f32r, kind="ExternalInput")
    ohd = nc.dram_tensor("onehot", [KS, N], bf16, kind="ExternalInput")
    wed = nc.dram_tensor("we", [12, NPL, 128], f32r, kind="ExternalInput")
    wsd = nc.dram_tensor("ws", [NPL // 8, KS, 8 * 128], bf16,
                         kind="ExternalInput")
    abd = nc.dram_tensor("ab", [128, 6], f16, kind="ExternalInput")
    outd = nc.dram_tensor("out", [2, 3, NPL, N], f32, kind="ExternalOutput")

    with tile.TileContext(nc) as tc:
        with (
            tc.tile_pool(name="io", bufs=1) as iop,
            tc.tile_pool(name="gauss", bufs=3) as gp,
            tc.tile_pool(name="trig", bufs=3) as trigp,
            tc.tile_pool(name="prod", bufs=3) as pp,
            tc.tile_pool(name="mms", bufs=2, space="PSUM") as mmsp,
            tc.tile_pool(name="mme", bufs=2, space="PSUM") as mmep,
        ):
            ab_sb = iop.tile([128, 6], f16, tag="ab")
            nc.scalar.dma_start(out=ab_sb[:], in_=abd[:])
            oh_sb = iop.tile([KS, N], bf16, tag="oh")
            nc.scalar.dma_start(out=oh_sb[:], in_=ohd[:])
            ft_sb = iop.tile([12, N], f32r, tag="ft")
            nc.scalar.dma_start(out=ft_sb[:], in_=featd[:])
            we = iop.tile([12, NPL, 128], f32r, tag="we")
            nc.scalar.dma_start(out=we[:], in_=wed[:])
            # PE interleaves wrap-gated mS singles with free-running mE
            # pairs; DVE wraps stream from ~4us (the critical chain); ACT
            # runs [all Exp-pairs][all Sins] - Sins floored past the last
            # Exp (Copy needs no table load). po tiles share the mS PSUM
            # ring (mS banks are dead once wrapped).
            wqs = []
            gts = []
            w1q = gq = mEp = None
            for pl in range(NPL):
                if pl == 0:
                    wsc = iop.tile([KS, 8, 128], bf16, tag="ws", bufs=2)
                    nc.sync.dma_start(out=wsc[:, 0:2, :],
                                      in_=wsd[0, :, 0:2 * 128])
                    nc.sync.dma_start(out=wsc[:, 2:8, :],
                                      in_=wsd[0, :, 2 * 128:])
                elif pl % 8 == 0:
                    wsc = iop.tile([KS, 8, 128], bf16, tag="ws", bufs=2)
                    nc.sync.dma_start(out=wsc[:], in_=wsd[pl // 8])
                if pl % 2 == 0:
                    mSp = mmsp.tile([128, 2, N], f32, tag="mS", name="mS")
                nc.tensor.matmul(mSp[:, pl % 2, :], wsc[:, pl % 8, :],
                                 oh_sb[:], start=True, stop=True)
                # two mE matmuls per mS: all Exps are done by mid-kernel
                for e in (2 * pl, 2 * pl + 1):
                    if e >= NPL:
                        continue
                    if e % 2 == 0:
                        mEp = mmep.tile([128, 2, N], f32, tag="mE",
                                        name="mE")
                    nc.tensor.matmul(mEp[:, e % 2, :], we[:, e, :],
                                     ft_sb[:], start=True, stop=True)
                    if e % 4 == 0:
                        gq = gp.tile([128, 4, N], f16, tag="g",
                                     name="gauss", bufs=NPL // 4 + 1)
                        gts.append(gq)
                    if e % 2 == 1:
                        nc.scalar.activation(gq[:, (e % 4) - 1:(e % 4) + 1],
                                             mEp[:], Act.Exp)
                if pl % 4 == 1:
                    w1q = trigp.tile([128, 4, N], f16, tag="w1", name="w1",
                                     bufs=NPL // 4 + 1)
                if pl % 2 == 1:
                    nc.vector.add_range_wrap(
                        w1q[:, (pl % 4) - 1:(pl % 4) + 1], mSp[:],
                        0.0, PI, 2.0 * PI)
                if pl % 4 == 3:
                    w2q = trigp.tile([128, 4, N], f16, tag="w2", name="w2",
                                     bufs=NPL // 4 + 1)
                    nc.vector.add_range_wrap(w2q[:], w1q[:],
                                             PI / 2, PI, 2.0 * PI)
                    wqs.append((w1q, w2q))

            for i in range(NPL // 4):
                w1q, w2q = wqs[i]
                gq = gts[i]
                with tc.tile_wait_until(SIN_FLOOR_MS):
                    ssq = trigp.tile([128, 4, N], f16, tag="ss", name="ss")
                    nc.scalar.activation(ssq[:], w1q[:], Act.Sin)
                    csq = trigp.tile([128, 4, N], f16, tag="cs", name="cs")
                    nc.scalar.activation(csq[:], w2q[:], Act.Sin)
                p1q = pp.tile([128, 4, N], f16, tag="p1", name="p1")
                nc.vector.tensor_mul(p1q[:], gq[:], csq[:])
                p2q = pp.tile([128, 4, N], f16, tag="p2", name="p2")
                nc.vector.tensor_mul(p2q[:], gq[:], ssq[:])
                for hh in range(2):       # pair within quad
                    _emit_reduce(nc, mmsp, pp, ab_sb, outd, p1q, p2q,
                                 2 * hh, 2 * i + hh)

    nc.compile()
    return nc


def _emit_reduce(nc, accp, pp, ab_sb, outd, p1q, p2q, hh, q):
    """2 logical tiles per PSUM bank at partition offsets 0/32 (base 96
    is illegal - PE quadrant 3), bank h = plane h of the pair; one
    accumulation group per (bank, region). PSUM->SBUF copies alternate
    between the Scalar and Vector engines to balance load."""
    from concourse import mybir
    f32 = mybir.dt.float32
    N_ = p1q.shape[-1]
    po = accp.tile([128, 2, N_], f32, tag="mS", name="po")
    for h in range(2):        # plane within pair = bank
        for s in range(2):    # side: 0=left(K 0:64) 1=right
            ks, co = s * 64, s * 32
            for pi_, (src, acol) in enumerate(((p1q, 0), (p2q, 3))):
                nc.tensor.matmul(
                    po[co:co + 3, h, :],
                    ab_sb[ks:ks + 64, acol:acol + 3],
                    src[ks:ks + 64, hh + h, :],
                    start=(pi_ == 0), stop=(pi_ == 1),
                    skip_group_check=True,
                )
    ob = pp.tile([128, 2, N_], f32, tag="ob", name="ob")
    if q % 2 == 0:
        nc.scalar.copy(ob[:], po[:])
    else:
        nc.vector.tensor_copy(ob[:], po[:])
    nc.sync.dma_start(out=outd[0, :, 2 * q:2 * q + 2, :], in_=ob[0:3, :, :])
    nc.sync.dma_start(out=outd[1, :, 2 * q:2 * q + 2, :], in_=ob[32:35, :, :])


def _wrap(x):
    return np.mod(x + np.pi, 2.0 * np.pi) - np.pi


def _to_f32r(a):
    b = np.ascontiguousarray(a, np.float32).view(np.uint32)
    r = (b + np.uint32(0x7FF) + ((b >> np.uint32(12)) & np.uint32(1))) \
        & np.uint32(0xFFFFF000)
    return r.view(np.float32)


def _to_bf16(a):
    import ml_dtypes
    return np.ascontiguousarray(a.astype(ml_dtypes.bfloat16))


def _fold_params(inputs):
    u = np.clip(np.asarray(inputs["u"], np.float64), -1, 1)
    v = np.clip(np.asarray(inputs["v"], np.float64), -1, 1)
    th = np.clip(np.asarray(inputs["theta"], np.float64), -2, 2) * (2 * np.pi)
    sig = np.clip(np.asarray(inputs["rel_sigma"], np.float64), 0.001, 1.0)
    rf = np.clip(np.asarray(inputs["rel_freq"], np.float64), -5, 5)
    gam = np.clip(np.asarray(inputs["gamma"], np.float64), 0.0001, 1.0)
    psi = np.clip(np.asarray(inputs["psi"], np.float64), -1, 1)
    amp = np.clip(np.asarray(inputs["amplitude"], np.float64), 0, 1)
    cr, sr = np.cos(th), np.sin(th)
    return dict(
        u=u, v=v, cr=cr, sr=sr,
        cx=-(cr * u + sr * v), cy=sr * u - cr * v,
        p=1.0 / (2.0 * sig * sig), q=1.0 / (2.0 * gam * gam),
        freq=2 * np.pi / np.exp(rf),
        alpha=amp * np.cos(psi * 2 * np.pi),
        beta=-amp * np.sin(psi * 2 * np.pi),
        amp=amp,
    )


def _keeps(P, gx, gy, rows, cols):
    """Exact per-pixel cull: keep gabors whose max E over the region
    clears the contribution threshold."""
    ampmax = P["amp"].max(1)
    elim = np.log(np.maximum(CULL_THR / np.maximum(ampmax, 1e-30),
                             1e-300)) - 1.0
    crf = P["cr"].astype(np.float32)[:, None]
    srf = P["sr"].astype(np.float32)[:, None]
    pf = P["p"].astype(np.float32)[:, None]
    qf = P["q"].astype(np.float32)[:, None]
    Xs = np.asarray(gx[rows][:, cols], np.float32).ravel()[None, :]
    Ys = np.asarray(gy[rows][:, cols], np.float32).ravel()[None, :]
    dx = Xs - P["u"].astype(np.float32)[:, None]
    dy = Ys - P["v"].astype(np.float32)[:, None]
    xr = dx * crf + dy * srf
    yr = dy * crf - dx * srf
    quad = xr * xr * pf
    quad += yr * yr * qf
    Em = -quad.min(1)
    return np.flatnonzero(Em >= elim)


def _tile_geometry(gx, gy):
    """Tile-major grids and per-tile affine centers/steps."""
    Xt = gx.reshape(H // TR, TR, W // TC, TC).transpose(0, 2, 1, 3).reshape(-1, N)
    Yt = gy.reshape(H // TR, TR, W // TC, TC).transpose(0, 2, 1, 3).reshape(-1, N)
    hx = Xt[:, 1] - Xt[:, 0]
    hy = Yt[:, TC] - Yt[:, 0]
    Xc = Xt[:, TR // 2 * TC + TC // 2]
    Yc = Yt[:, TR // 2 * TC + TC // 2]
    yrow = Yt.reshape(-1, TR, TC)[:, :, 0]
    xcol = Xt.reshape(-1, TR, TC)[:, 0, :]
    return Xc, Yc, hx, hy, yrow, xcol


def _tile_tables(P, keep, tiles, Xc, Yc, hx, hy, yrow, xcol):
    """WE [6, n, k], A [n, k, TR], B [n, k, TC] for the given gabor subset
    over the given tile indices (float64)."""
    crk, srk = P["cr"][keep], P["sr"][keep]
    cxk, cyk = P["cx"][keep], P["cy"][keep]
    pk, qk = P["p"][keep], P["q"][keep]
    fk = P["freq"][keep]
    XcT = Xc[tiles][:, None]
    YcT = Yc[tiles][:, None]
    hxT = hx[tiles][:, None]
    hyT = hy[tiles][:, None]
    cxt = XcT * crk[None, :] + YcT * srk[None, :] + cxk[None, :]
    cyt = -XcT * srk[None, :] + YcT * crk[None, :] + cyk[None, :]
    a1 = hxT * crk[None, :]
    a2 = hyT * srk[None, :]
    b1 = -hxT * srk[None, :]
    b2 = hyT * crk[None, :]
    n, k = cxt.shape
    WE = np.empty((6, n, k))
    WE[0] = -2.0 * (pk * cxt * a1 + qk * cyt * b1)
    WE[1] = -2.0 * (pk * cxt * a2 + qk * cyt * b2)
    WE[2] = -(pk * cxt * cxt + qk * cyt * cyt)
    WE[3] = -(pk * a1 * a1 + qk * b1 * b1)
    WE[4] = -(pk * a2 * a2 + qk * b2 * b2)
    WE[5] = -2.0 * (pk * a1 * a2 + qk * b1 * b2)
    A = _wrap(fk[None, :, None] * srk[None, :, None]
              * (yrow[tiles][:, None, :] - YcT[:, :, None]))
    Bt = _wrap(fk[None, :, None] * crk[None, :, None]
               * (xcol[tiles][:, None, :] - XcT[:, :, None])
               + (fk[None, :] * cxt)[:, :, None])
    return WE, A, Bt


def _host_arrays_packed(inputs, P, gx, gy, keepLR):
    ii, jj = np.divmod(np.arange(N), TC)
    di = (ii - TR // 2).astype(np.float64)
    dj = (jj - TC // 2).astype(np.float64)
    feat6 = np.stack([dj, di, np.ones_like(dj), dj * dj, di * di, dj * di], 0)
    feat12 = np.concatenate([feat6, feat6], 0).astype(np.float32)

    onehot = np.zeros((KS, N), np.float32)
    onehot[ii, np.arange(N)] = 1.0
    onehot[TR + jj, np.arange(N)] = 1.0
    onehot[TR + TC:] = onehot[:TR + TC]

    Xc, Yc, hx, hy, yrow, xcol = _tile_geometry(gx, gy)

    # plane pl = r*8+cb -> left tile r*16+cb, right tile r*16+cb+8
    rr = np.arange(NPL) // 8
    cc = np.arange(NPL) % 8

    in_maps = []
    for core in range(NCORES):
        keepL, keepR = keepLR[core]
        base = core * NT
        tilesL = base + rr * 16 + cc
        tilesR = tilesL + 8

        we12 = np.zeros((12, NPL, 128), np.float32)
        ws = np.zeros((KS, NPL, 128))
        AB = np.zeros((128, 6))
        for side, keep, tiles in (
            (0, keepL, tilesL), (1, keepR, tilesR)
        ):
            k = len(keep)
            o = side * 64
            WE, A, Bt = _tile_tables(P, keep, tiles, Xc, Yc, hx, hy,
                                     yrow, xcol)
            WEh = _to_f32r(WE)
            WEl = _to_f32r(WE - WEh)
            we12[0:6, :, o:o + k] = WEh
            we12[6:12, :, o:o + k] = WEl
            WS = np.concatenate([A.transpose(2, 0, 1),
                                 Bt.transpose(2, 0, 1)], 0)  # [48, NPL, k]
            WSh = _to_bf16(WS).astype(np.float64)
            ws[0:48, :, o:o + k] = WSh
            ws[48:96, :, o:o + k] = WS - WSh
            AB[o:o + k, 0:3] = P["alpha"][keep]
            AB[o:o + k, 3:6] = P["beta"][keep]

        ws_cm = ws.reshape(KS, NPL // 8, 8 * 128).transpose(1, 0, 2)
        in_maps.append({
            "feat": feat12,
            "onehot": _to_bf16(onehot),
            "we": np.ascontiguousarray(we12),
            "ws": _to_bf16(ws_cm),
            "ab": AB.astype(np.float16),
        })
    return in_maps


def kernel(**inputs):
    from concourse.bass_utils import run_bass_kernel_spmd

    gx = np.asarray(inputs["grid_x"], np.float64)
    gy = np.asarray(inputs["grid_y"], np.float64)
    P = _fold_params(inputs)

    keepLR = []
    packed = True
    for core in range(NCORES):
        rows = slice(core * SH, (core + 1) * SH)
        kL = _keeps(P, gx, gy, rows, slice(0, W // 2))
        kR = _keeps(P, gx, gy, rows, slice(W // 2, W))
        if len(kL) > 64 or len(kR) > 64:
            packed = False
        keepLR.append((kL, kR))

    if not packed:
        return _kernel_unpacked(inputs)

    in_maps = _host_arrays_packed(inputs, P, gx, gy, keepLR)
    if "packed" not in _PROGRAMS:
        _PROGRAMS["packed"] = _build_program_packed()
    nc = _PROGRAMS["packed"]
    res = run_bass_kernel_spmd(nc, in_maps, list(range(NCORES)))
    out = np.empty((3, H, W), np.float32)
    for core in range(NCORES):
        r = res.results[core]["out"]              # [2, 3, NPL, N]
        # plane pl = rowblk*8+cb; side 0 -> tile col cb, side 1 -> cb+8
        arr = r.reshape(2, 3, SH // TR, 8, TR, TC)
        out[:, core * SH:(core + 1) * SH, :] = (
            arr.transpose(1, 2, 4, 0, 3, 5).reshape(3, SH, W)
        )
    np.clip(out, -1.0, 1.0, out=out)
    return out


# ---------------------------------------------------------------------------
# Fallback: v2 per-tile program (used only if a column half keeps > 64
# gabors; correct for any input).
# ---------------------------------------------------------------------------

B_FB = 8


def _build_program_unpacked(nchunk):
    from concourse import bacc, mybir, tile

    f32 = mybir.dt.float32
    f32r = mybir.dt.float32r
    bf16 = mybir.dt.bfloat16
    f16 = mybir.dt.float16
    Act = mybir.ActivationFunctionType
    Gc = 128 * nchunk
    mmbufs = 2 if nchunk == 1 else 1
    NBLK = NT // B_FB

    nc = bacc.Bacc("TRN2", target_bir_lowering=False, debug=False,
                   num_devices=NCORES)

    featd = nc.dram_tensor("feat", [12, N], f32r, kind="ExternalInput")
    ohd = nc.dram_tensor("onehot", [KS, N], bf16, kind="ExternalInput")
    wed = nc.dram_tensor("we", [12, NT, Gc], f32r, kind="ExternalInput")
    wsd = nc.dram_tensor("ws", [KS, NT, Gc], bf16, kind="ExternalInput")
    abd = nc.dram_tensor("ab", [128, nchunk * 2 * 3], f16,
                         kind="ExternalInput")
    outd = nc.dram_tensor("out", [3, NT, N], f32, kind="ExternalOutput")

    with tile.TileContext(nc) as tc:
        with (
            tc.tile_pool(name="io", bufs=1) as iop,
            tc.tile_pool(name="gauss", bufs=B_FB // 2 + 2) as gp,
            tc.tile_pool(name="trig", bufs=3) as trigp,
            tc.tile_pool(name="prod", bufs=3) as pp,
            tc.tile_pool(name="mme", bufs=mmbufs, space="PSUM") as mmep,
            tc.tile_pool(name="mms", bufs=mmbufs, space="PSUM") as mmsp,
            tc.tile_pool(name="acc", bufs=2, space="PSUM") as accp,
        ):
            ab_sb = iop.tile([128, nchunk * 2 * 3], f16, tag="ab")
            nc.sync.dma_start(out=ab_sb[:], in_=abd[:])
            oh_sb = iop.tile([KS, N], bf16, tag="oh")
            nc.sync.dma_start(out=oh_sb[:], in_=ohd[:])
            ft_sb = iop.tile([12, N], f32r, tag="ft")
            nc.sync.dma_start(out=ft_sb[:], in_=featd[:])

            for blk in range(NBLK):
                t0 = blk * B_FB
                we = iop.tile([12, B_FB, Gc], f32r, tag="we", bufs=2)
                nc.sync.dma_start(out=we[:], in_=wed[:, t0:t0 + B_FB, :])
                ws = iop.tile([KS, B_FB, Gc], bf16, tag="ws", bufs=2)
                nc.sync.dma_start(out=ws[:], in_=wsd[:, t0:t0 + B_FB, :])

                gts = []
                for t in range(B_FB):
                    mE = mmep.tile([128, nchunk, N], f32, tag="mE", name="mE")
                    for c in range(nchunk):
                        nc.tensor.matmul(
                            mE[:, c, :],
                            we[:, t, c * 128:(c + 1) * 128],
                            ft_sb[:],
                            start=True, stop=True,
                        )
                    if t % 2 == 0:
                        gpair = gp.tile([128, 2 * nchunk, N], f16, tag="g",
                                        name="gauss")
                        gts.append(gpair)
                    nc.scalar.activation(
                        gpair[:, (t % 2) * nchunk:(t % 2 + 1) * nchunk],
                        mE[:], Act.Exp)

                for t in range(B_FB):
                    mS = mmsp.tile([128, nchunk, N], f32, tag="mS", name="mS")
                    for c in range(nchunk):
                        nc.tensor.matmul(
                            mS[:, c, :],
                            ws[:, t, c * 128:(c + 1) * 128],
                            oh_sb[:],
                            start=True, stop=True,
                        )
                    if t % 2 == 0:
                        w1p = trigp.tile([128, 2 * nchunk, N], f16, tag="w1",
                                         name="w1")
                    nc.vector.add_range_wrap(
                        w1p[:, (t % 2) * nchunk:(t % 2 + 1) * nchunk],
                        mS[:], 0.0, PI, 2.0 * PI)
                    if t % 2 == 1:
                        w2p = trigp.tile([128, 2 * nchunk, N], f16, tag="w2",
                                         name="w2")
                        nc.vector.add_range_wrap(w2p[:], w1p[:],
                                                 PI / 2, PI, 2.0 * PI)
                        ssp = trigp.tile([128, 2 * nchunk, N], f16, tag="ss",
                                         name="ss")
                        nc.scalar.activation(ssp[:], w1p[:], Act.Sin)
                        csp = trigp.tile([128, 2 * nchunk, N], f16, tag="cs",
                                         name="cs")
                        nc.scalar.activation(csp[:], w2p[:], Act.Sin)

                        gpair = gts[t // 2]
                        p1p = pp.tile([128, 2 * nchunk, N], f16, tag="p1",
                                      name="p1")
                        nc.vector.tensor_mul(p1p[:], gpair[:], csp[:])
                        p2p = pp.tile([128, 2 * nchunk, N], f16, tag="p2",
                                      name="p2")
                        nc.vector.tensor_mul(p2p[:], gpair[:], ssp[:])

                        po = accp.tile([3, 2, N], f32, tag="po", name="po")
                        for hh in range(2):
                            ops = [(p1p, c) for c in range(nchunk)] + \
                                  [(p2p, c) for c in range(nchunk)]
                            for ci, (src, c) in enumerate(ops):
                                ab_col = (0 if src is p1p
                                          else 3 * nchunk) + 3 * c
                                nc.tensor.matmul(
                                    po[:, hh],
                                    ab_sb[:, ab_col:ab_col + 3],
                                    src[:, hh * nchunk + c, :],
                                    start=(ci == 0),
                                    stop=(ci == len(ops) - 1),
                                )
                        ob = pp.tile([3, 2, N], f32, tag="ob", name="ob")
                        nc.vector.tensor_copy(ob[:], po[:])
                        nc.sync.dma_start(
                            out=outd[:, t0 + t - 1:t0 + t + 1, :],
                            in_=ob[:],
                        )

    nc.compile()
    return nc


def _kernel_unpacked(inputs):
    from concourse.bass_utils import run_bass_kernel_spmd

    gx = np.asarray(inputs["grid_x"], np.float64)
    gy = np.asarray(inputs["grid_y"], np.float64)
    P = _fold_params(inputs)

    keep_lists = []
    for core in range(NCORES):
        rows = slice(core * SH, (core + 1) * SH)
        keep_lists.append(_keeps(P, gx, gy, rows, slice(0, W)))
    gmax = max(len(k) for k in keep_lists)
    nchunk = max(1, -(-gmax // 128))
    Gc = 128 * nchunk

    ii, jj = np.divmod(np.arange(N), TC)
    di = (ii - TR // 2).astype(np.float64)
    dj = (jj - TC // 2).astype(np.float64)
    feat6 = np.stack([dj, di, np.ones_like(dj), dj * dj, di * di, dj * di], 0)
    feat12 = np.concatenate([feat6, feat6], 0).astype(np.float32)

    onehot = np.zeros((KS, N), np.float32)
    onehot[ii, np.arange(N)] = 1.0
    onehot[TR + jj, np.arange(N)] = 1.0
    onehot[TR + TC:] = onehot[:TR + TC]

    Xc, Yc, hx, hy, yrow, xcol = _tile_geometry(gx, gy)

    in_maps = []
    for core in range(NCORES):
        keep = keep_lists[core]
        k = len(keep)
        tiles = np.arange(core * NT, (core + 1) * NT)
        WE, A, Bt = _tile_tables(P, keep, tiles, Xc, Yc, hx, hy, yrow, xcol)

        we12 = np.zeros((12, NT, Gc), np.float32)
        WEh = _to_f32r(WE)
        we12[0:6, :, :k] = WEh
        we12[6:12, :, :k] = _to_f32r(WE - WEh)

        ws = np.zeros((KS, NT, Gc))
        WS = np.concatenate([A.transpose(2, 0, 1), Bt.transpose(2, 0, 1)], 0)
        WSh = _to_bf16(WS).astype(np.float64)
        ws[0:48, :, :k] = WSh
        ws[48:96, :, :k] = WS - WSh

        AB = np.zeros((128, nchunk * 2 * 3))
        al = np.zeros((Gc, 3)); bt = np.zeros((Gc, 3))
        al[:k] = P["alpha"][keep]
        bt[:k] = P["beta"][keep]
        for c in range(nchunk):
            AB[:, 3 * c:3 * c + 3] = al[c * 128:(c + 1) * 128]
            off = 3 * (nchunk + c)
            AB[:, off:off + 3] = bt[c * 128:(c + 1) * 128]

        in_maps.append({
            "feat": feat12,
            "onehot": _to_bf16(onehot),
            "we": np.ascontiguousarray(we12),
            "ws": _to_bf16(ws),
            "ab": AB.astype(np.float16),
        })

    key = ("unpacked", nchunk)
    if key not in _PROGRAMS:
        _PROGRAMS[key] = _build_program_unpacked(nchunk)
    nc = _PROGRAMS[key]
    res = run_bass_kernel_spmd(nc, in_maps, list(range(NCORES)))
    out = np.empty((3, H, W), np.float32)
    for core in range(NCORES):
        r = res.results[core]["out"]              # [3, NT, N]
        out[:, core * SH:(core + 1) * SH, :] = (
            r.reshape(3, SH // TR, TPR, TR, TC)
             .transpose(0, 1, 3, 2, 4)
             .reshape(3, SH, W)
        )
    np.clip(out, -1.0, 1.0, out=out)
    return out
